# revision 18
# baseline (speedup 1.0000x reference)
"""Trainium2 Bass kernel for ExpBertSelfAttention (B=2, S=2048, D=1024, H=16).

Sharding: 8 cores; core c handles batch b=c//4 and 4 consecutive heads
4*(c%4)..4*(c%4)+3 (data-parallel on B, tensor-parallel on heads), as 2 head
pairs.  The dense output projection is row-parallel: each core returns a
partial [S, D] sum (bf16); the host adds the 4 partials per batch + b_dense.

Per-core data path (all matmuls bf16, f32 PSUM accumulation; softmax scale
1/sqrt(hd) folded into Wq on host; no max-subtraction — scores are O(1) by
construction):

  - hsT [D, S] / wqkv [D, 768] / maskT [S, S] / wd [256, D] arrive bf16.
  - QKV: qkvT[768, S] = wqkv^T hsT, kt-accumulated in PSUM, drained (+bias)
    to bf16 SBUF by the Pool engine.  Pair-0 m-tiles run in the lead-in with
    6 PSUM banks; pair-1 runs on a 1-bank ping-pong interleaved into pair-0's
    attention so the PE never idles waiting for projection.
  - V is transposed to [key, hd] per 128-key tile by the XBAR DMA transpose
    (dma_start_transpose, [64,128]->[128,64]), with a constant bf16 ones
    column at index 64 so the PV matmul also emits the softmax row-sum.
  - scores^T[k, q] per (kt, head): K=64 matmul from qkvT slices; exp on ACT
    (PSUM f32 -> SBUF bf16); multiplicative {0,1} bf16 mask applied by DVE at
    its 2x 16-bit rate (exact).
  - PV is *flipped*: ctx^T[q, 65] = pt[:,qtile]^T @ [V | 1], accumulated over
    kt into sub-bank PSUM slots ([128, 6, 80] f32 tiles, one bank each).
    M=128 (query) is fully used, so PV costs 65 PE rows per (qtile, kt)
    instead of 2x512 — half the naive cost — and the row-sum lands as a
    per-partition scalar, making normalization a native tensor_scalar_mul
    (no cross-partition broadcast matmuls).
  - normalize: reciprocal_approx_fast on the 16 gathered row-sum columns,
    tensor_scalar_mul into a [q, 128] staging tile (two heads side by side),
    then one XBAR DMA transpose [128,128] per qtile lands ctx directly in
    the [head-pair-hd, q] layout the dense matmul wants.
  - dense: y[qtile, D] accumulated over the 2 pairs in a 1-bank PSUM
    ping-pong, drained to bf16 by Pool, streamed out per qtile.

Engine budget per core (TimelineSim cost model): PE ~137us (329k rows @2.4GHz)
and ACT ~133us (128 exps of [128,1024] @ 1038ns) are the co-roofs; DVE ~90us,
Pool ~55us, DMA ~60us all fit underneath.
"""

import os
import sys

for _p in ("/opt/trn_rl_repo", "/root/.axon_site/_ro/trn_rl_repo"):
    if os.path.isdir(_p) and _p not in sys.path:
        sys.path.insert(0, _p)

import numpy as np
import ml_dtypes

import concourse.bass as bass
import concourse.tile as tile
from concourse import bacc, mybir
from concourse import bass_utils

B, S, D, H = 2, 2048, 1024, 16
HD = D // H  # 64
SCALE = float(np.sqrt(HD).astype(np.float32))
NCORES = 8
HPC = H // (NCORES // B)  # heads per core = 4
P = 128
F32 = mybir.dt.float32
BF16 = mybir.dt.bfloat16
AF = mybir.ActivationFunctionType

KT_HS = D // P            # 8 contraction tiles for QKV
KT_S = S // P             # 16 key tiles for attention
QC = 1024                 # q chunk
NQC = S // QC             # 2
NQT = QC // P             # 8 query tiles per chunk
VW = 80                   # v_sb slot stride (65 used: 64 v + ones); must
                          # keep XBAR-transpose dst offsets 32B-aligned


def build_program():
    nc = bacc.Bacc("TRN2", target_bir_lowering=False, debug=False,
                   num_devices=NCORES)

    hsT = nc.dram_tensor("hsT", [D, S], BF16, kind="ExternalInput").ap()
    wqkv = nc.dram_tensor("wqkv", [D, 3 * HPC * HD], BF16,
                          kind="ExternalInput").ap()
    bqkv = nc.dram_tensor("bqkv", [3 * HPC * HD], F32,
                          kind="ExternalInput").ap()
    maskT = nc.dram_tensor("maskT", [S, S], BF16, kind="ExternalInput").ap()
    wd = nc.dram_tensor("wd", [2 * P, D], BF16, kind="ExternalInput").ap()
    y = nc.dram_tensor("y", [S, D], BF16, kind="ExternalOutput").ap()
    dbg = os.environ.get("BK_DEBUG", "") == "1"
    if dbg:
        d_qkvT = nc.dram_tensor("d_qkvT", [P, 6, S], BF16,
                                kind="ExternalOutput").ap()
        d_v = nc.dram_tensor("d_v", [P, 2, KT_S, 2, VW], BF16,
                             kind="ExternalOutput").ap()
        d_cp = nc.dram_tensor("d_cp", [P, 2, S], BF16,
                              kind="ExternalOutput").ap()
        d_pt = nc.dram_tensor("d_pt", [P, QC], BF16,
                              kind="ExternalOutput").ap()
        d_rs = nc.dram_tensor("d_rs", [P, 16], F32,
                              kind="ExternalOutput").ap()
        d_ri = nc.dram_tensor("d_ri", [P, 16], F32,
                              kind="ExternalOutput").ap()
        d_cn = nc.dram_tensor("d_cn", [P, NQT, P], BF16,
                              kind="ExternalOutput").ap()

    hsT_r = hsT.rearrange("(t p) n -> p t n", p=P)
    w_r = wqkv.rearrange("(t p) n -> p t n", p=P)

    with tile.TileContext(nc) as tc:
        with tc.tile_pool(name="persist", bufs=1) as persist:
            hsT_sb = persist.tile([P, KT_HS, S], BF16)          # 32 KB/part
            w_sb = persist.tile([P, KT_HS, 3 * HPC * HD], BF16)  # 12 KB/part
            bq_sb = persist.tile([P, 6], F32)
            qkvT = persist.tile([P, 6, S], BF16)                # 24 KB/part
            # v slots [pr, kt, hl, VW]: cols 0-63 = V^T, col 64 = ones
            v_sb = persist.tile([P, 2, KT_S, 2, VW], BF16)      # 9 KB/part
            wd_sb = persist.tile([P, 2, D], BF16)               # 4 KB/part
            ctx_pair = persist.tile([P, 2, S], BF16)            # 8 KB/part
            ones_f = persist.tile([P, HD], F32)

            nc.sync.dma_start(bq_sb[:], bqkv.rearrange("(t p) -> p t", p=P))
            nc.vector.memset(ones_f[:], 1.0)
            nc.vector.tensor_copy(
                v_sb[:, :, :, :, HD:HD + 1].rearrange(
                    "p a b c d -> p (a b c d)"),
                ones_f[:, 0:2 * KT_S * 2])

            # ------------- Phase 1: QKV projection, head-pair 0 -------------
            # Streams (w_kt, hsT_kt) DMA pairs; 3 PSUM accumulators (m-tiles
            # 0/2/4 = Q/K/V of pair 0), double-buffered across 512-col chunks.
            with tc.tile_pool(name="p1ps", bufs=2, space="PSUM") as p1ps:
                # load order: (w_kt, hsT_kt cols 0:512) pairs first so the
                # nch-0 accumulation chain is DMA-complete early, then the
                # remaining hsT columns
                for kt in range(KT_HS):
                    nc.sync.dma_start(w_sb[:, kt, :], w_r[:, kt, :])
                    nc.sync.dma_start(hsT_sb[:, kt, 0:512],
                                      hsT_r[:, kt, 0:512])
                for kt in range(KT_HS):
                    nc.sync.dma_start(hsT_sb[:, kt, 512:S],
                                      hsT_r[:, kt, 512:S])
                for nch in range(S // 512):
                    cs = slice(nch * 512, (nch + 1) * 512)
                    ps_l = {mt: p1ps.tile([P, 512], F32, tag=f"q{mt}",
                                          name=f"qkv_ps{mt}_{nch}")
                            for mt in (0, 2, 4)}
                    for kt in range(KT_HS):
                        for mt in (0, 2, 4):
                            nc.tensor.matmul(
                                ps_l[mt][:],
                                w_sb[:, kt, mt * P:(mt + 1) * P],
                                hsT_sb[:, kt, cs],
                                start=(kt == 0), stop=(kt == KT_HS - 1))
                    for mt in (0, 2, 4):
                        # ACT drain: lead-in, the Activation engine is idle
                        # (GPSIMD cannot access PSUM on TRN2)
                        nc.scalar.add(qkvT[:, mt, cs], ps_l[mt][:],
                                      bq_sb[:, mt:mt + 1])
                # V^T via XBAR DMA transpose: [64, 128] -> [128, 64]
                for kt in range(KT_S):
                    for hl in range(2):
                        nc.sync.dma_start_transpose(
                            v_sb[:, 0, kt, hl, 0:HD],
                            qkvT[hl * HD:(hl + 1) * HD, 4,
                                 kt * P:(kt + 1) * P])

            # ------------- Phase 2+3: attention + dense -------------
            with (
                tc.tile_pool(name="mp", bufs=2) as mp,
                tc.tile_pool(name="ptp", bufs=4) as ptp,
                tc.tile_pool(name="cnp", bufs=2) as cnp,
                tc.tile_pool(name="rp", bufs=2) as rp,
                tc.tile_pool(name="yp", bufs=2) as yp,
                tc.tile_pool(name="sps", bufs=2, space="PSUM") as sps,
                tc.tile_pool(name="cps", bufs=1, space="PSUM") as cps,
                tc.tile_pool(name="scr", bufs=1, space="PSUM") as scrp,
            ):
                # 1-bank ping-pong shared by QKV pair-1 and dense
                scr = scrp.tile([P, 2, 256], F32)
                scr_i = [0]

                def qkv1_run(mt, ch):
                    sl = scr_i[0] % 2
                    scr_i[0] += 1
                    cs = slice(ch * 256, (ch + 1) * 256)
                    for kt in range(KT_HS):
                        nc.tensor.matmul(
                            scr[:, sl, :],
                            w_sb[:, kt, mt * P:(mt + 1) * P],
                            hsT_sb[:, kt, cs],
                            start=(kt == 0), stop=(kt == KT_HS - 1))
                    nc.vector.tensor_scalar_add(
                        qkvT[:, mt, cs], scr[:, sl, :], bq_sb[:, mt:mt + 1])

                def dense_qt(qc, qt):
                    q0 = qc * QC + qt * P
                    y_t = yp.tile([P, D], BF16, tag="y")
                    for ch in range(4):
                        sl = scr_i[0] % 2
                        scr_i[0] += 1
                        cs = slice(ch * 256, (ch + 1) * 256)
                        for pr2 in range(2):
                            nc.tensor.matmul(
                                scr[:, sl, :],
                                ctx_pair[:, pr2, q0:q0 + P],
                                wd_sb[:, pr2, cs],
                                start=(pr2 == 0), stop=(pr2 == 1))
                        nc.vector.tensor_copy(y_t[:, cs], scr[:, sl, :])
                    nc.sync.dma_start(y[q0:q0 + P, :], y_t[:])

                # fill-work queue: (emitted interleaved into attention kt
                # loops so the in-order PE stream overlaps them with the
                # ACT-paced attention)
                fill = [("qkv1", mt, ch)
                        for mt in (3, 1, 5) for ch in range(8)]
                nc.sync.dma_start(wd_sb[:],
                                  wd.rearrange("(t p) n -> p t n", p=P))

                def pop_fill(n):
                    for _ in range(n):
                        if not fill:
                            return
                        item = fill.pop(0)
                        if item[0] == "qkv1":
                            qkv1_run(item[1], item[2])
                            if item[1] == 5:
                                # chunk ch covers key tiles 2ch, 2ch+1
                                for kt in (2 * item[2], 2 * item[2] + 1):
                                    for hl in range(2):
                                        nc.sync.dma_start_transpose(
                                            v_sb[:, 1, kt, hl, 0:HD],
                                            qkvT[hl * HD:(hl + 1) * HD, 5,
                                                 kt * P:(kt + 1) * P])
                        else:
                            dense_qt(item[1], item[2])

                for qc in range(2):
                    q0 = qc * QC
                    mask_t = mp.tile([P, KT_S, QC], BF16, tag="mask")
                    for g in range(8):
                        nc.sync.dma_start(
                            mask_t[:, 2 * g:2 * g + 2, :],
                            maskT[g * 256:(g + 1) * 256,
                                  q0:q0 + QC].rearrange(
                                      "(t p) q -> p t q", p=P))
                    for pr in range(2):
                        ctx_t = [cps.tile([P, 6, 80], F32, tag=f"ctx{t}",
                                          name=f"ctx{t}_{pr}_{qc}")
                                 for t in range(3)]
                        # 6 accumulation slots share each PSUM bank, and a
                        # start=True matmul clears has_written bits bank-wide
                        # (clobbering sibling slots' accumulation state).  So:
                        # zero the banks with DVE and accumulate with
                        # start=False throughout — correct whether the prior
                        # bits are set (accumulate onto 0) or clear (replace
                        # the 0 with the kt-0 term).
                        for t in range(3):
                            nc.vector.memset(
                                ctx_t[t][:].rearrange("p a b -> p (a b)"),
                                0.0)
                        for kt in range(KT_S):
                            for hl in range(2):
                                rows = slice(hl * HD, (hl + 1) * HD)
                                s_ps = sps.tile([P, QC], F32, tag="s")
                                for ch in range(2):
                                    cs = slice(ch * 512, (ch + 1) * 512)
                                    nc.tensor.matmul(
                                        s_ps[:, cs],
                                        qkvT[rows, 2 + pr,
                                             kt * P:(kt + 1) * P],
                                        qkvT[rows, 0 + pr,
                                             q0 + ch * 512:
                                             q0 + (ch + 1) * 512],
                                        start=True, stop=True)
                                pt = ptp.tile([P, QC], BF16, tag="pt")
                                nc.scalar.activation(pt[:], s_ps[:], AF.Exp)
                                if dbg and qc == 0 and pr == 0 and kt == 0 \
                                        and hl == 0:
                                    nc.sync.dma_start(d_pt, pt[:])
                                # ~1/3 of the (SBUF-only, bf16) mask
                                # multiplies go to the otherwise-idle GPSIMD
                                if (2 * kt + hl) % 3 == 2:
                                    nc.gpsimd.tensor_mul(pt[:], pt[:],
                                                         mask_t[:, kt, :])
                                else:
                                    nc.vector.tensor_mul(pt[:], pt[:],
                                                         mask_t[:, kt, :])
                                for qt in range(NQT):
                                    s = hl * NQT + qt
                                    t, sl = divmod(s, 6)
                                    nc.tensor.matmul(
                                        ctx_t[t][:, sl, 0:HD + 1],
                                        pt[:, qt * P:(qt + 1) * P],
                                        v_sb[:, pr, kt, hl, 0:HD + 1],
                                        start=False,
                                        stop=(kt == KT_S - 1),
                                        skip_group_check=True)
                            pop_fill(2)
                        # normalize: row-sums sit at free-col 64 as
                        # per-partition (per-q) scalars
                        rsum = rp.tile([P, 2 * NQT], F32, tag="rs")
                        for t, (lo, n) in enumerate(((0, 6), (6, 6),
                                                     (12, 4))):
                            nc.vector.tensor_copy(
                                rsum[:, lo:lo + n],
                                ctx_t[t][:, 0:n, HD:HD + 1].rearrange(
                                    "p a b -> p (a b)"))
                        rinv = rp.tile([P, 2 * NQT], F32, tag="ri")
                        nc.vector.reciprocal_approx_fast(rinv[:], rsum[:])
                        if dbg and qc == 0 and pr == 0:
                            nc.sync.dma_start(d_rs, rsum[:])
                            nc.sync.dma_start(d_ri, rinv[:])
                        ctxn = cnp.tile([P, NQT, P], BF16, tag="cn")
                        for hl in range(2):
                            for qt in range(NQT):
                                s = hl * NQT + qt
                                t, sl = divmod(s, 6)
                                nc.vector.tensor_scalar_mul(
                                    ctxn[:, qt, hl * HD:(hl + 1) * HD],
                                    ctx_t[t][:, sl, 0:HD],
                                    rinv[:, s:s + 1])
                        if dbg and qc == 0 and pr == 0:
                            nc.sync.dma_start(d_cn, ctxn[:])
                        for qt in range(NQT):
                            nc.sync.dma_start_transpose(
                                ctx_pair[:, pr, q0 + qt * P:q0 + (qt + 1) * P],
                                ctxn[:, qt, :])
                    # queue dense for this q chunk as fill work (qc 0);
                    # the final chunk runs in the tail
                    fill.extend([("dense", qc, qt) for qt in range(NQT)])
                pop_fill(len(fill))
                if dbg:
                    nc.sync.dma_start(d_qkvT, qkvT[:])
                    nc.sync.dma_start(d_v, v_sb[:])
                    nc.sync.dma_start(d_cp, ctx_pair[:])

    nc.compile()
    return nc


_NC = None


def get_program():
    global _NC
    if _NC is None:
        _NC = build_program()
    return _NC


def make_in_maps(hidden_states, attention_mask, W_qkv, b_qkv, W_dense, b_dense):
    hs = np.asarray(hidden_states, np.float32)
    mask = np.asarray(attention_mask)
    W_qkv = np.asarray(W_qkv, np.float32)
    b_qkv = np.asarray(b_qkv, np.float32)
    W_dense = np.asarray(W_dense, np.float32)

    hsT = [np.ascontiguousarray(hs[b].T).astype(ml_dtypes.bfloat16)
           for b in range(B)]
    maskT = [np.ascontiguousarray(
        np.where(mask[b, 0], 1.0, 0.0).astype(np.float32).T).astype(
            ml_dtypes.bfloat16) for b in range(B)]

    Wq, Wk, Wv = W_qkv[:, :D], W_qkv[:, D:2 * D], W_qkv[:, 2 * D:]
    bq, bk, bv = b_qkv[:D], b_qkv[D:2 * D], b_qkv[2 * D:]

    in_maps = []
    for c in range(NCORES):
        b = c // (NCORES // B)
        h0 = HPC * (c % (NCORES // B))
        cols = slice(h0 * HD, (h0 + HPC) * HD)
        wqkv_c = np.concatenate(
            [Wq[:, cols] / SCALE, Wk[:, cols], Wv[:, cols]], axis=1)
        bqkv_c = np.concatenate(
            [bq[cols] / SCALE, bk[cols], bv[cols]]).astype(np.float32)
        in_maps.append({
            "hsT": hsT[b],
            "wqkv": np.ascontiguousarray(wqkv_c).astype(ml_dtypes.bfloat16),
            "bqkv": bqkv_c,
            "maskT": maskT[b],
            "wd": np.ascontiguousarray(W_dense[cols, :]).astype(
                ml_dtypes.bfloat16),
        })
    return in_maps


def kernel(hidden_states, attention_mask, W_qkv, b_qkv, W_dense, b_dense,
           **run_kwargs):
    nc = get_program()
    in_maps = make_in_maps(hidden_states, attention_mask, W_qkv, b_qkv,
                           W_dense, b_dense)
    res = bass_utils.run_bass_kernel_spmd(
        nc, in_maps, core_ids=list(range(NCORES)), **run_kwargs)
    out = np.zeros((B, S, D), np.float32)
    gpb = NCORES // B
    for c in range(NCORES):
        out[c // gpb] += np.asarray(res.results[c]["y"], np.float32)
    out += np.asarray(b_dense, np.float32)
    if run_kwargs:
        kernel.last_results = res
    return out


# revision 20
# speedup vs baseline: 1.0482x; 1.0482x over previous
"""Trainium2 Bass kernel for ExpBertSelfAttention (B=2, S=2048, D=1024, H=16).

Sharding: 8 cores; core c handles batch b=c//4 and 4 consecutive heads
4*(c%4)..4*(c%4)+3 (data-parallel on B, tensor-parallel on heads).  The dense
output projection is row-parallel: each core returns a partial [S, D] sum
(bf16); the host adds the 4 partials per batch + b_dense.

Per-core data path (all matmuls bf16 with f32 PSUM; 1/sqrt(hd) folded into
Wq on host; softmax without max-subtraction — scores are O(1) by
construction):

  - QKV: qkvT[768, S] = wqkv^T hsT.  Pair-0 m-tiles run in the lead-in;
    pair-1 runs as 8-matmul accumulation chains interleaved into the
    attention kt loops (PE fill work), both through a shared 3-slot
    [128, 1024] PSUM ring.  Drains: ACT (lead-in) / DVE (in-flight).
  - V is transposed to [key, hd] via XBAR DMA transpose ([64,128]->[128,64]
    into 80-element slots: dst offsets must stay 32B-aligned), with a
    constant ones column at index 64 so PV also emits the softmax row-sum.
  - Attention is processed one head at a time (8 sections = 4 heads x 2
    q-chunks); the ctx accumulator [65, 1024] then needs only 2 PSUM banks.
    Per kt: QK (2 x N=512 matmuls into a ring slot), one [128,1024] exp on
    ACT (psum f32 -> sbuf bf16), multiplicative {0,1} bf16 mask on DVE (2x
    16-bit rate) or GPSIMD (1 in 3, SBUF-only), PV (2 x N=512 into ctx).
  - normalize: row-sum row 64 -> ACT copy to SBUF -> k=1 matmul broadcast
    across partitions (into a ring slot) -> reciprocal_approx_fast (DVE) ->
    multiply into the pair-stacked bf16 ctx_pair; odd heads reach
    partitions 64-127 via a small partition-shifting SBUF->SBUF DMA.
  - dense: y[qtile, D] accumulated over the 2 pairs through ring slots,
    DVE-drained to bf16, streamed out per qtile; the qc-0 half overlaps the
    qc-1 attention.

Engine budget per core (TimelineSim model): PE.ENGINE ~167us (401k rows
@2.4GHz) is the roof; PE.SEQ ~144us (880 Ldweights+Matmult pairs), ACT
~141us (128 exps + row copies), DVE ~107us, Pool ~87us, DMA ~62us fit under.
"""

import os
import sys

for _p in ("/opt/trn_rl_repo", "/root/.axon_site/_ro/trn_rl_repo"):
    if os.path.isdir(_p) and _p not in sys.path:
        sys.path.insert(0, _p)

import numpy as np
import ml_dtypes

import concourse.bass as bass
import concourse.tile as tile
from concourse import bacc, mybir
from concourse import bass_utils

B, S, D, H = 2, 2048, 1024, 16
HD = D // H  # 64
SCALE = float(np.sqrt(HD).astype(np.float32))
NCORES = 8
HPC = H // (NCORES // B)  # heads per core = 4
P = 128
F32 = mybir.dt.float32
BF16 = mybir.dt.bfloat16
AF = mybir.ActivationFunctionType

KT_HS = D // P            # 8 contraction tiles for QKV
KT_S = S // P             # 16 key tiles for attention
QC = 1024                 # q chunk
NQC = S // QC             # 2
NQT = QC // P             # 8 query tiles per chunk
VW = 80                   # v_sb slot stride (65 used: 64 v + ones column);
                          # XBAR-transpose dst offsets must be 32B-aligned


def build_program():
    nc = bacc.Bacc("TRN2", target_bir_lowering=False, debug=False,
                   num_devices=NCORES)

    hsT = nc.dram_tensor("hsT", [D, S], BF16, kind="ExternalInput").ap()
    wqkv = nc.dram_tensor("wqkv", [D, 3 * HPC * HD], BF16,
                          kind="ExternalInput").ap()
    bqkv = nc.dram_tensor("bqkv", [3 * HPC * HD], F32,
                          kind="ExternalInput").ap()
    maskT = nc.dram_tensor("maskT", [S, S], BF16, kind="ExternalInput").ap()
    wd = nc.dram_tensor("wd", [2 * P, D], BF16, kind="ExternalInput").ap()
    y = nc.dram_tensor("y", [S, D], BF16, kind="ExternalOutput").ap()
    dbg = os.environ.get("BK_DEBUG", "") == "1"
    if dbg:
        d_qkvT = nc.dram_tensor("d_qkvT", [P, 6, S], BF16,
                                kind="ExternalOutput").ap()
        d_v = nc.dram_tensor("d_v", [P, HPC, KT_S, VW], BF16,
                             kind="ExternalOutput").ap()
        d_cp = nc.dram_tensor("d_cp", [P, 2, S], BF16,
                              kind="ExternalOutput").ap()

    hsT_r = hsT.rearrange("(t p) n -> p t n", p=P)
    w_r = wqkv.rearrange("(t p) n -> p t n", p=P)

    with tile.TileContext(nc) as tc:
        with tc.tile_pool(name="persist", bufs=1) as persist:
            hsT_sb = persist.tile([P, KT_HS, S], BF16)          # 32 KB/part
            w_sb = persist.tile([P, KT_HS, 3 * HPC * HD], BF16)  # 12 KB/part
            bq_sb = persist.tile([P, 6], F32)
            qkvT = persist.tile([P, 6, S], BF16)                # 24 KB/part
            # v slots [h, kt, VW]: cols 0-63 = V^T, col 64 = ones
            v_sb = persist.tile([P, HPC, KT_S, VW], BF16)       # 10 KB/part
            wd_sb = persist.tile([P, 2, D], BF16)               # 4 KB/part
            ctx_pair = persist.tile([P, 2, S], BF16)            # 8 KB/part
            ones_f = persist.tile([P, HD], F32)
            ones_bf = persist.tile([P, HD], BF16)

            nc.sync.dma_start(bq_sb[:], bqkv.rearrange("(t p) -> p t", p=P))
            nc.vector.memset(ones_f[:], 1.0)
            nc.vector.tensor_copy(ones_bf[:], ones_f[:])
            nc.vector.tensor_copy(
                v_sb[:, :, :, HD:HD + 1].rearrange("p a b c -> p (a b c)"),
                ones_f[:, 0:HPC * KT_S])

            with (
                tc.tile_pool(name="mp", bufs=2) as mp,
                tc.tile_pool(name="ptp", bufs=4) as ptp,
                tc.tile_pool(name="rp", bufs=2) as rp,
                tc.tile_pool(name="yp", bufs=2) as yp,
                tc.tile_pool(name="sps", bufs=3, space="PSUM") as sps,
                tc.tile_pool(name="cps", bufs=1, space="PSUM") as cps,
            ):
                # ---------------- Phase 1: QKV pair 0 ----------------
                # (w_kt, hsT_kt cols 0:512) DMA pairs stream first so the
                # first accumulation chains are DMA-complete early.
                for kt in range(KT_HS):
                    nc.sync.dma_start(w_sb[:, kt, :], w_r[:, kt, :])
                    nc.sync.dma_start(hsT_sb[:, kt, 0:512],
                                      hsT_r[:, kt, 0:512])
                for kt in range(KT_HS):
                    nc.sync.dma_start(hsT_sb[:, kt, 512:S],
                                      hsT_r[:, kt, 512:S])

                def qkv_chain(mt, nch, drain):
                    """one [128,512] column chunk of qkvT m-tile mt through
                    a ring slot; drain = 'act' (lead-in) or 'dve'"""
                    cs = slice(nch * 512, (nch + 1) * 512)
                    ps = sps.tile([P, 512], F32, tag="s",
                                  name=f"qkv_ps{mt}_{nch}")
                    for kt in range(KT_HS):
                        nc.tensor.matmul(
                            ps[:], w_sb[:, kt, mt * P:(mt + 1) * P],
                            hsT_sb[:, kt, cs],
                            start=(kt == 0), stop=(kt == KT_HS - 1))
                    if drain == "act":
                        nc.scalar.add(qkvT[:, mt, cs], ps[:],
                                      bq_sb[:, mt:mt + 1])
                    else:
                        nc.vector.tensor_scalar_add(
                            qkvT[:, mt, cs], ps[:], bq_sb[:, mt:mt + 1])

                def v_transposes(pr, nch):
                    # V m-tile (4+pr) 512-col chunk nch covers key tiles
                    # 4nch .. 4nch+3
                    for kt in range(4 * nch, 4 * nch + 4):
                        for hl in range(2):
                            nc.sync.dma_start_transpose(
                                v_sb[:, 2 * pr + hl, kt, 0:HD],
                                qkvT[hl * HD:(hl + 1) * HD, 4 + pr,
                                     kt * P:(kt + 1) * P])

                for nch in range(4):
                    for mt in (0, 2, 4):
                        qkv_chain(mt, nch, "act")
                    v_transposes(0, nch)

                nc.sync.dma_start(wd_sb[:],
                                  wd.rearrange("(t p) n -> p t n", p=P))

                # -------- Phase 2+3: attention + interleaved fill --------
                fill = [("qkv1", mt, nch)
                        for nch in range(4) for mt in (3, 1, 5)]

                def dense_qt(qc, qt):
                    q0 = qc * QC + qt * P
                    y_t = yp.tile([P, D], BF16, tag="y")
                    for ch in range(2):
                        cs = slice(ch * 512, (ch + 1) * 512)
                        ps = sps.tile([P, 512], F32, tag="s",
                                      name=f"d_{qc}_{qt}_{ch}")
                        for pr2 in range(2):
                            nc.tensor.matmul(
                                ps[:], ctx_pair[:, pr2, q0:q0 + P],
                                wd_sb[:, pr2, cs],
                                start=(pr2 == 0), stop=(pr2 == 1))
                        nc.vector.tensor_copy(y_t[:, cs], ps[:])
                    nc.sync.dma_start(y[q0:q0 + P, :], y_t[:])

                def pop_fill(n):
                    for _ in range(n):
                        if not fill:
                            return
                        item = fill.pop(0)
                        if item[0] == "qkv1":
                            qkv_chain(item[1], item[2], "dve")
                            if item[1] == 5:
                                v_transposes(1, item[2])
                        else:
                            dense_qt(item[1], item[2])

                for qc in range(NQC):
                    q0 = qc * QC
                    mask_t = mp.tile([P, KT_S, QC], BF16, tag="mask")
                    for g in range(8):
                        nc.sync.dma_start(
                            mask_t[:, 2 * g:2 * g + 2, :],
                            maskT[g * 256:(g + 1) * 256,
                                  q0:q0 + QC].rearrange(
                                      "(t p) q -> p t q", p=P))
                    for h in range(HPC):
                        pr, hl = divmod(h, 2)
                        rows = slice(hl * HD, (hl + 1) * HD)
                        ctx_ps = cps.tile([HD + 1, QC], F32, tag="ctx",
                                          name=f"ctx_{h}_{qc}")
                        for kt in range(KT_S):
                            s_ps = sps.tile([P, QC], F32, tag="s",
                                            name=f"s_{h}_{qc}_{kt}")
                            for ch in range(2):
                                cs = slice(ch * 512, (ch + 1) * 512)
                                nc.tensor.matmul(
                                    s_ps[:, cs],
                                    qkvT[rows, 2 + pr, kt * P:(kt + 1) * P],
                                    qkvT[rows, 0 + pr,
                                         q0 + ch * 512:q0 + (ch + 1) * 512],
                                    start=True, stop=True)
                            pt = ptp.tile([P, QC], BF16, tag="pt")
                            nc.scalar.activation(pt[:], s_ps[:], AF.Exp)
                            if kt % 3 == 2:
                                nc.gpsimd.tensor_mul(pt[:], pt[:],
                                                     mask_t[:, kt, :])
                            else:
                                nc.vector.tensor_mul(pt[:], pt[:],
                                                     mask_t[:, kt, :])
                            for ch in range(2):
                                cs = slice(ch * 512, (ch + 1) * 512)
                                nc.tensor.matmul(
                                    ctx_ps[:, cs],
                                    v_sb[:, h, kt, 0:HD + 1],
                                    pt[:, cs],
                                    start=(kt == 0), stop=(kt == KT_S - 1))
                            pop_fill(1)
                        # ---- normalize head h ----
                        # row-sum lives on psum partition 64; broadcast it
                        # across partitions with a k=1 matmul (ring slot),
                        # then 1/x on DVE and multiply into ctx_pair.
                        rr = rp.tile([HD + 1, QC], BF16, tag="rr")
                        nc.scalar.copy(rr[HD:HD + 1, :],
                                       ctx_ps[HD:HD + 1, :])
                        rb = sps.tile([HD, QC], F32, tag="s",
                                      name=f"rb_{h}_{qc}")
                        for ch in range(2):
                            cs = slice(ch * 512, (ch + 1) * 512)
                            nc.tensor.matmul(
                                rb[:, cs], ones_bf[HD:HD + 1, :],
                                rr[HD:HD + 1, cs], start=True, stop=True)
                        rbi = rp.tile([HD, QC], F32, tag="rbi")
                        nc.vector.reciprocal_approx_fast(rbi[:], rb[:])
                        if hl == 0:
                            nc.vector.tensor_mul(
                                ctx_pair[0:HD, pr, q0:q0 + QC],
                                ctx_ps[0:HD, :], rbi[:])
                        else:
                            # engines cannot cross partitions: stage on
                            # partitions 0-63, partition-shift with DMA
                            stg = rp.tile([HD, QC], BF16, tag="stg")
                            nc.vector.tensor_mul(stg[:], ctx_ps[0:HD, :],
                                                 rbi[:])
                            nc.sync.dma_start(
                                ctx_pair[HD:P, pr, q0:q0 + QC], stg[:])
                    fill.extend([("dense", qc, qt) for qt in range(NQT)])
                pop_fill(len(fill))
                if dbg:
                    nc.sync.dma_start(d_qkvT, qkvT[:])
                    nc.sync.dma_start(d_v, v_sb[:])
                    nc.sync.dma_start(d_cp, ctx_pair[:])

    nc.compile()
    return nc


_NC = None


def get_program():
    global _NC
    if _NC is None:
        _NC = build_program()
    return _NC


def make_in_maps(hidden_states, attention_mask, W_qkv, b_qkv, W_dense, b_dense):
    hs = np.asarray(hidden_states, np.float32)
    mask = np.asarray(attention_mask)
    W_qkv = np.asarray(W_qkv, np.float32)
    b_qkv = np.asarray(b_qkv, np.float32)
    W_dense = np.asarray(W_dense, np.float32)

    hsT = [np.ascontiguousarray(hs[b].T).astype(ml_dtypes.bfloat16)
           for b in range(B)]
    maskT = [np.ascontiguousarray(
        np.where(mask[b, 0], 1.0, 0.0).astype(np.float32).T).astype(
            ml_dtypes.bfloat16) for b in range(B)]

    Wq, Wk, Wv = W_qkv[:, :D], W_qkv[:, D:2 * D], W_qkv[:, 2 * D:]
    bq, bk, bv = b_qkv[:D], b_qkv[D:2 * D], b_qkv[2 * D:]

    in_maps = []
    for c in range(NCORES):
        b = c // (NCORES // B)
        h0 = HPC * (c % (NCORES // B))
        cols = slice(h0 * HD, (h0 + HPC) * HD)
        wqkv_c = np.concatenate(
            [Wq[:, cols] / SCALE, Wk[:, cols], Wv[:, cols]], axis=1)
        bqkv_c = np.concatenate(
            [bq[cols] / SCALE, bk[cols], bv[cols]]).astype(np.float32)
        in_maps.append({
            "hsT": hsT[b],
            "wqkv": np.ascontiguousarray(wqkv_c).astype(ml_dtypes.bfloat16),
            "bqkv": bqkv_c,
            "maskT": maskT[b],
            "wd": np.ascontiguousarray(W_dense[cols, :]).astype(
                ml_dtypes.bfloat16),
        })
    return in_maps


def kernel(hidden_states, attention_mask, W_qkv, b_qkv, W_dense, b_dense,
           **run_kwargs):
    nc = get_program()
    in_maps = make_in_maps(hidden_states, attention_mask, W_qkv, b_qkv,
                           W_dense, b_dense)
    res = bass_utils.run_bass_kernel_spmd(
        nc, in_maps, core_ids=list(range(NCORES)), **run_kwargs)
    out = np.zeros((B, S, D), np.float32)
    gpb = NCORES // B
    for c in range(NCORES):
        out[c // gpb] += np.asarray(res.results[c]["y"], np.float32)
    out += np.asarray(b_dense, np.float32)
    if run_kwargs:
        kernel.last_results = res
    return out
